# revision 25
# baseline (speedup 1.0000x reference)
"""GAT kernel for Trainium2 (Bass/Tile), data-parallel over batch on 8 cores.

Per-core math (one batch element, N nodes, H heads, D=E=128). Softmax is
invariant to per-row scaling, so the exp(0.8*a_s_i) row factor of the exact
exp'd scores cancels against the normalizer and the attention weights can
be computed as

  p[j,i] = max(v2_j, s23_j * u3_i) * adjT[j,i]
    u3_i  = exp(0.8*a_s_i)        (host)
    s23_j = exp(a_n_j - 2)        (host)
    v2_j  = exp(0.2*a_n_j - 2)    (host)

since exp(lrelu(z,0.2)) = exp(0.2 z)*exp(0.8 relu(z)) and, after dropping
the cancelling row factor, exp(0.2 a_n_j - 2)*max(1, e^{0.8 z}) =
max(v2_j, s23_j*u3_i) with z = a_s_i + a_n_j.  u3 arrives from the host
pre-broadcast across all 128 partitions (U3B), and s23/v2 arrive as
per-partition scalar columns, so per (head, j-chunk) the device does

  q = tensor_scalar(U3B, s23_c, v2_c, mult, max)     (DVE, one fused op)
  p = q * adjT_chunk          (DVE tensor_tensor, batched 4 chunks/op)
  acc_ib += p[:, ib-block]^T @ [feat | 1]   for all 8 i-blocks (PE, bf16,
            all-incremental PSUM accumulation across 8 banks; the rowsum
            falls out of the ones column)
  out[i,:] = relu(acc * (1/rowsum))   (ACT relu with per-partition scale,
            reciprocal on DVE; feat copies PSUM->SBUF also ride on ACT)

No N^2 work ever touches the scalar engine and nothing round-trips PSUM
except the matmul accumulators themselves; the scalar engine only does the
small feat copies and the epilogue, and GPSIMD is unused (its elementwise
throughput and SBUF-port sharing with the DVE make offloading a net loss).
"""

import sys

sys.path.insert(0, "/opt/trn_rl_repo")

import numpy as np
import ml_dtypes

import concourse.bacc as bacc
import concourse.mybir as mybir
import concourse.tile as tile
from concourse.bass_utils import run_bass_kernel_spmd

F32 = mybir.dt.float32
F16 = mybir.dt.bfloat16
NP16 = ml_dtypes.bfloat16
P = 128


def build_core_program(N, H, D=128, E=128):
    """Trace the Bass program computing one batch element of the GAT."""
    nc = bacc.Bacc("TRN2", debug=False, target_bir_lowering=False)
    NCH = N // P   # node chunks
    EA = E + 1     # feat columns + ones column
    HP = H // 2    # head pairs

    # wx = [kw (H*E cols, head-major) | xT (N cols)] packed on host
    WXW = H * E + N
    wx = nc.dram_tensor("wx", [D, WXW], F16, kind="ExternalInput").ap()
    adjT = nc.dram_tensor("adjT", [N, N], F16, kind="ExternalInput").ap()
    # u3 rows broadcast across partitions, head-major blocks of 128 rows
    u3b = nc.dram_tensor("u3b", [H * P, N], F16, kind="ExternalInput").ap()
    # per-partition scalars: col (c*H+h)*2 = s23, +1 = v2 (fp32)
    svm = nc.dram_tensor("svm", [P, NCH * H * 2], F32,
                         kind="ExternalInput").ap()
    out = nc.dram_tensor("out", [N, H * E], F16, kind="ExternalOutput").ap()

    with tile.TileContext(nc) as tc:
        with (
            tc.tile_pool(name="xt", bufs=1) as xt_pool,
            tc.tile_pool(name="u3b", bufs=1) as u3b_pool,
            tc.tile_pool(name="svt", bufs=1) as svt_pool,
            tc.tile_pool(name="adj", bufs=1) as adj_pool,
            tc.tile_pool(name="fr", bufs=1) as fr_pool,
        ):
            # fs: [P, NCH*H*(E+1)] bf16, chunk-major then head-major
            # [feat_h | 1] blocks; ones columns written by one ACT const op
            # so the DVE queue has no startup work.
            fsall = fr_pool.tile([P, NCH * H * EA], F16, tag="fs")
            fs = [fsall[:, c * H * EA:(c + 1) * H * EA] for c in range(NCH)]

            wx_sb = xt_pool.tile([D, WXW], F16, tag="wx")
            kw_sb = wx_sb[:, 0:H * E]
            xt_sb = wx_sb[:, H * E:WXW]
            svm_sb = svt_pool.tile([P, NCH * H * 2], F32, tag="svm")
            nc.scalar.activation(
                fsall[:].rearrange("p (x f) -> p x f", f=EA)[:, :, E],
                svm_sb[:, 0:NCH * H],
                mybir.ActivationFunctionType.Copy, bias=1.0, scale=0.0)
            u3b_sb = [u3b_pool.tile([P, N], F16, tag=f"u3b{h}",
                                    name=f"u3b{h}") for h in range(H)]
            # adj as chunk-quad tiles [P, 4N] so the mask multiply can run
            # one DVE op per quad
            QC = 4 if NCH % 4 == 0 else 2
            NG = NCH // QC
            adj2_sb = [adj_pool.tile([P, QC * N], F16, tag=f"adj{g}",
                                     name=f"adj{g}") for g in range(NG)]

            # DMA order tuned for startup: head 0 needs svm, u3b[0] and the
            # first adj chunks before anything else; wx (proj) next; the
            # rest stream behind.
            nc.sync.dma_start(out=svm_sb[:], in_=svm[:])
            nc.sync.dma_start(out=u3b_sb[0][:], in_=u3b[0:P, :])
            for k in range(QC):
                nc.sync.dma_start(out=adj2_sb[0][:, k * N:(k + 1) * N],
                                  in_=adjT[k * P:(k + 1) * P, :])
            nc.sync.dma_start(out=wx_sb[:, 0:H * E + N // 2],
                              in_=wx[:, 0:H * E + N // 2])
            nc.sync.dma_start(out=wx_sb[:, H * E + N // 2:WXW],
                              in_=wx[:, H * E + N // 2:WXW])
            for g in range(1, NG):
                for k in range(QC):
                    c = g * QC + k
                    nc.sync.dma_start(out=adj2_sb[g][:, k * N:(k + 1) * N],
                                      in_=adjT[c * P:(c + 1) * P, :])
                if g < H:
                    nc.sync.dma_start(out=u3b_sb[g][:],
                                      in_=u3b[g * P:(g + 1) * P, :])
            for h in range(max(1, NG), H):
                nc.sync.dma_start(out=u3b_sb[h][:],
                                  in_=u3b[h * P:(h + 1) * P, :])

            # ---- PSUM: proj 1 bank + 7 accumulator banks = 8 (bufs is
            # per-tag, so acc_ps holds NCH-1 single-buffered slots).
            with (
                tc.tile_pool(name="proj_ps", bufs=1, space="PSUM") as proj_ps,
                tc.tile_pool(name="acc_ps", bufs=1, space="PSUM") as acc_ps,
            ):
                # projection: all H heads in one rhs (H*E <= 512 cols)
                for c in range(NCH):
                    ps = proj_ps.tile([P, H * E], F32, tag="proj")
                    nc.tensor.matmul(
                        ps[:],
                        xt_sb[:, c * P:(c + 1) * P],
                        kw_sb,
                        start=True, stop=True,
                    )
                    nc.scalar.copy(
                        fs[c][:].rearrange("p (h f) -> p h f", h=H)[:, :, 0:E],
                        ps[:].rearrange("p (h f) -> p h f", h=H))

                # ---- per-head attention ----
                with (
                    tc.tile_pool(name="q", bufs=2) as q_pool,
                    tc.tile_pool(name="p", bufs=3) as p_pool,
                    tc.tile_pool(name="ep", bufs=4) as ep_pool,
                ):
                    for h in range(H):
                        # NCH concurrent accumulators: i-blocks 0..NCH-1
                        accs = []
                        for ib in range(NCH):
                            if ib < NCH - 1:
                                accs.append(acc_ps.tile(
                                    [P, EA], F32, tag=f"acc{ib}",
                                    name=f"acc{h}_{ib}"))
                            else:
                                accs.append(proj_ps.tile(
                                    [P, EA], F32, tag="proj",
                                    name=f"acc{h}_{ib}"))
                        for g in range(NG):
                            q2 = q_pool.tile([P, QC * N], F16, tag="q",
                                             name=f"q{h}_{g}")
                            for k in range(QC):
                                c = g * QC + k
                                sc = (c * H + h) * 2
                                nc.vector.tensor_scalar(
                                    out=q2[:, k * N:(k + 1) * N],
                                    in0=u3b_sb[h][:],
                                    scalar1=svm_sb[:, sc:sc + 1],
                                    scalar2=svm_sb[:, sc + 1:sc + 2],
                                    op0=mybir.AluOpType.mult,
                                    op1=mybir.AluOpType.max)
                            p2 = p_pool.tile([P, QC * N], F16, tag="p",
                                             name=f"p{h}_{g}")
                            # split the final group's mask op so the stop-
                            # matmuls (and the epilogue) start half a quad
                            # sooner; the whole last head runs fine-grained
                            # to shorten the kernel tail
                            halves = 2 if QC >= 2 and (
                                g == NG - 1 or h == H - 1) else 1
                            W = QC * N // halves
                            for hv in range(halves):
                                nc.vector.tensor_tensor(
                                    p2[:, hv * W:(hv + 1) * W],
                                    q2[:, hv * W:(hv + 1) * W],
                                    adj2_sb[g][:, hv * W:(hv + 1) * W],
                                    mybir.AluOpType.mult)
                            for k in range(QC):
                                c = g * QC + k
                                for ib in range(NCH):
                                    nc.tensor.matmul(
                                        accs[ib][:],
                                        p2[:, k * N + ib * P:
                                           k * N + (ib + 1) * P],
                                        fs[c][:, h * EA:(h + 1) * EA],
                                        start=(c == 0), stop=(c == NCH - 1),
                                    )

                        # epilogue: out = relu(acc / rowsum), ones-col rowsum
                        HB = NCH // 2
                        for ib in range(NCH):
                            acc = accs[ib]
                            rec = ep_pool.tile([P, 1], F32, tag="rec",
                                               name=f"rec{h}_{ib}")
                            nc.vector.reciprocal(rec[:], acc[:, E:E + 1])
                            if ib == 0:
                                obh = [ep_pool.tile([P, HB * E], F16,
                                                    tag=f"obh{half}", bufs=2,
                                                    name=f"obh{h}_{half}")
                                       for half in range(2)]
                            nc.scalar.activation(
                                obh[ib // HB][:, (ib % HB) * E:
                                              (ib % HB + 1) * E],
                                acc[:, 0:E],
                                mybir.ActivationFunctionType.Relu,
                                bias=0.0, scale=rec[:])
                        # two DMAs per head (i-block halves):
                        # partition r, free (ib, c) -> row ib*P+r, col h*E+c
                        for half in range(2):
                            nc.sync.dma_start(
                                out=out[half * HB * P:(half + 1) * HB * P,
                                        h * E:(h + 1) * E].rearrange(
                                    "(ib r) c -> r ib c", r=P),
                                in_=obh[half][:].rearrange(
                                    "p (ib c) -> p ib c", c=E))
    nc.compile()
    return nc


_PROGRAM_CACHE = {}


def _get_program(N, H):
    key = (N, H)
    if key not in _PROGRAM_CACHE:
        _PROGRAM_CACHE[key] = build_core_program(N, H)
    return _PROGRAM_CACHE[key]


def host_prep(x, adj, kernel, attn_self, attn_neigh):
    """Build per-core input maps (layout transforms + small vector math)."""
    B, N, D = x.shape
    H, _, E = kernel.shape
    NCH = N // P
    kas = np.stack([kernel[h] @ attn_self[h] for h in range(H)], 1)  # [D,H]
    kan = np.stack([kernel[h] @ attn_neigh[h] for h in range(H)], 1)
    kw = np.concatenate([kernel[h] for h in range(H)], axis=1)  # [D, H*E]
    in_maps = []
    for b in range(B):
        a_s = x[b] @ kas   # [N, H]
        a_n = x[b] @ kan
        u3 = np.exp(0.8 * a_s.T)                  # [H, N]
        s23 = np.exp(a_n.T - 2.0)
        v2 = np.exp(0.2 * a_n.T - 2.0)
        u3b = np.repeat(u3.astype(NP16), P, axis=0)   # [H*P, N] broadcast
        # svm[r, (c*H+h)*2 + {0,1}] = {s23, v2}[h, c*128+r]
        sv = np.stack([s23, v2], -1)                  # [H, N, 2]
        svm = sv.transpose(1, 0, 2).reshape(NCH, P, H * 2)
        svm = svm.transpose(1, 0, 2).reshape(P, NCH * H * 2)
        wx = np.concatenate([kw, x[b].T], axis=1)
        in_maps.append({
            "wx": np.ascontiguousarray(wx).astype(NP16),
            "adjT": np.ascontiguousarray(adj[b].T).astype(NP16),
            "u3b": np.ascontiguousarray(u3b),
            "svm": np.ascontiguousarray(svm).astype(np.float32),
        })
    return in_maps


def kernel(x, adj, kernel, attn_self, attn_neigh, bias, _profile=None):
    x = np.asarray(x, np.float32)
    adj = np.asarray(adj, np.float32)
    kernel = np.asarray(kernel, np.float32)
    attn_self = np.asarray(attn_self, np.float32)
    attn_neigh = np.asarray(attn_neigh, np.float32)
    bias = np.asarray(bias, np.float32)

    B, N, D = x.shape
    H, _, E = kernel.shape
    nc = _get_program(N, H)
    in_maps = host_prep(x, adj, kernel, attn_self, attn_neigh)
    kwargs = dict(_profile) if _profile else {}
    last_err = None
    for _attempt in range(3):
        try:
            res = run_bass_kernel_spmd(nc, in_maps, list(range(B)), **kwargs)
            outs = np.stack(
                [np.asarray(res.results[b]["out"]).astype(np.float32)
                 for b in range(B)])
            if np.isnan(outs).any() or np.isinf(outs).any():
                last_err = RuntimeError("non-finite kernel output")
                continue
            break
        except Exception as exc:  # transient PJRT/axon fetch errors
            last_err = exc
    else:
        raise last_err
    assert not np.any(bias != 0.0), "nonzero-bias path not implemented"
    if _profile:
        return outs, res
    return outs


if __name__ == "__main__":
    # Mini smoke test: N=256, H=2, B=2 against a numpy reference.
    np.random.seed(0)
    N, H, D, E, B = 256, 2, 128, 128, 2
    x = np.random.randn(B, N, D).astype(np.float32)
    adj = (np.random.rand(B, N, N) < 0.5).astype(np.float32)
    K = (np.random.randn(H, D, E) / np.sqrt(D)).astype(np.float32)
    a_s = (np.random.randn(H, E) / np.sqrt(E)).astype(np.float32)
    a_n = (np.random.randn(H, E) / np.sqrt(E)).astype(np.float32)
    bias = np.zeros((H, E), np.float32)

    def ref(x, adj, K, a_s, a_n, bias):
        feat = np.einsum('bnd,hde->bhne', x, K)
        s1 = np.einsum('bhne,he->bhn', feat, a_s)
        s2 = np.einsum('bhne,he->bhn', feat, a_n)
        sc = s1[..., :, None] + s2[..., None, :]
        sc = np.where(sc > 0, sc, 0.2 * sc)
        sc = sc + (-1e10) * (1.0 - adj[:, None])
        sc = sc - sc.max(axis=-1, keepdims=True)
        att = np.exp(sc)
        att = att / att.sum(axis=-1, keepdims=True)
        o = np.einsum('bhnm,bhme->bhne', att, feat) + bias[None, :, None, :]
        o = o.transpose(0, 2, 1, 3).reshape(B, N, H * E)
        return np.maximum(o, 0.0)

    expected = ref(x, adj, K, a_s, a_n, bias)
    nc = _get_program(N, H)
    in_maps = host_prep(x, adj, K, a_s, a_n)
    res = run_bass_kernel_spmd(nc, in_maps, list(range(B)))
    actual = np.stack([np.asarray(res.results[b]["out"]).astype(np.float32)
                       for b in range(B)])
    err = np.abs(actual - expected).max() / np.abs(expected).max()
    rel = np.linalg.norm(actual - expected) / np.linalg.norm(expected)
    print(f"SMOKE absmax-rel: {err:.3e}  l2-rel: {rel:.3e}")


# revision 26
# speedup vs baseline: 1.1293x; 1.1293x over previous
"""GAT kernel for Trainium2 (Bass/Tile), data-parallel over batch on 8 cores.

Per-core math (one batch element, N nodes, H heads, D=E=128). Softmax is
invariant to per-row scaling, so the exp(0.8*a_s_i) row factor of the exact
exp'd scores cancels against the normalizer and the attention weights can
be computed as

  p[j,i] = max(v2_j, s23_j * u3_i) * adjT[j,i]
    u3_i  = exp(0.8*a_s_i)        (host)
    s23_j = exp(a_n_j - 2)        (host)
    v2_j  = exp(0.2*a_n_j - 2)    (host)

since exp(lrelu(z,0.2)) = exp(0.2 z)*exp(0.8 relu(z)) and, after dropping
the cancelling row factor, exp(0.2 a_n_j - 2)*max(1, e^{0.8 z}) =
max(v2_j, s23_j*u3_i) with z = a_s_i + a_n_j.  u3 arrives from the host
pre-broadcast across all 128 partitions (U3B), and s23/v2 arrive as
per-partition scalar columns, so per (head, j-chunk) the device does

  q = tensor_scalar(U3B, s23_c, v2_c, mult, max)     (DVE, one fused op)
  p = q * adjT_chunk          (DVE tensor_tensor, batched 4 chunks/op)
  acc_ib += p[:, ib-block]^T @ [feat | 1]   for all 8 i-blocks (PE, bf16,
            all-incremental PSUM accumulation across 8 banks; the rowsum
            falls out of the ones column)
  out[i,:] = relu(acc * (1/rowsum))   (ACT relu with per-partition scale,
            reciprocal on DVE; feat copies PSUM->SBUF also ride on ACT)

No N^2 work ever touches the scalar engine and nothing round-trips PSUM
except the matmul accumulators themselves; the scalar engine only does the
small feat copies and the epilogue, and GPSIMD is unused (its elementwise
throughput and SBUF-port sharing with the DVE make offloading a net loss).
"""

import sys

sys.path.insert(0, "/opt/trn_rl_repo")

import numpy as np
import ml_dtypes

import concourse.bacc as bacc
import concourse.mybir as mybir
import concourse.tile as tile
from concourse.bass_utils import run_bass_kernel_spmd

F32 = mybir.dt.float32
F16 = mybir.dt.bfloat16
NP16 = ml_dtypes.bfloat16
P = 128


def build_core_program(N, H, D=128, E=128):
    """Trace the Bass program computing one batch element of the GAT."""
    nc = bacc.Bacc("TRN2", debug=False, target_bir_lowering=False)
    NCH = N // P   # node chunks
    EA = E + 1     # feat columns + ones column
    HP = H // 2    # head pairs

    # wx = [kw (H*E cols, head-major) | xT (N cols)] packed on host
    WXW = H * E + N
    wx = nc.dram_tensor("wx", [D, WXW], F16, kind="ExternalInput").ap()
    adjT = nc.dram_tensor("adjT", [N, N], F16, kind="ExternalInput").ap()
    # u3 rows broadcast across partitions, head-major blocks of 128 rows
    u3b = nc.dram_tensor("u3b", [H * P, N], F16, kind="ExternalInput").ap()
    # per-partition scalars: col (c*H+h)*2 = s23, +1 = v2 (fp32)
    svm = nc.dram_tensor("svm", [P, NCH * H * 2], F32,
                         kind="ExternalInput").ap()
    out = nc.dram_tensor("out", [N, H * E], F16, kind="ExternalOutput").ap()

    with tile.TileContext(nc) as tc:
        with (
            tc.tile_pool(name="xt", bufs=1) as xt_pool,
            tc.tile_pool(name="u3b", bufs=1) as u3b_pool,
            tc.tile_pool(name="svt", bufs=1) as svt_pool,
            tc.tile_pool(name="adj", bufs=1) as adj_pool,
            tc.tile_pool(name="fr", bufs=1) as fr_pool,
        ):
            # fs: [P, NCH*H*(E+1)] bf16, chunk-major then head-major
            # [feat_h | 1] blocks; ones columns written by one ACT const op
            # so the DVE queue has no startup work.
            fsall = fr_pool.tile([P, NCH * H * EA], F16, tag="fs")
            fs = [fsall[:, c * H * EA:(c + 1) * H * EA] for c in range(NCH)]

            wx_sb = xt_pool.tile([D, WXW], F16, tag="wx")
            kw_sb = wx_sb[:, 0:H * E]
            xt_sb = wx_sb[:, H * E:WXW]
            svm_sb = svt_pool.tile([P, NCH * H * 2], F32, tag="svm")
            nc.scalar.activation(
                fsall[:].rearrange("p (x f) -> p x f", f=EA)[:, :, E],
                svm_sb[:, 0:NCH * H],
                mybir.ActivationFunctionType.Copy, bias=1.0, scale=0.0)
            u3b_sb = [u3b_pool.tile([P, N], F16, tag=f"u3b{h}",
                                    name=f"u3b{h}") for h in range(H)]
            # adj as chunk-quad tiles [P, 4N] so the mask multiply can run
            # one DVE op per quad
            QC = 4 if NCH % 4 == 0 else 2
            NG = NCH // QC
            adj2_sb = [adj_pool.tile([P, QC * N], F16, tag=f"adj{g}",
                                     name=f"adj{g}") for g in range(NG)]

            # DMA order tuned for startup: head 0 needs svm, u3b[0] and the
            # first adj chunks before anything else; wx (proj) next; the
            # rest stream behind.
            nc.sync.dma_start(out=svm_sb[:], in_=svm[:])
            nc.sync.dma_start(out=u3b_sb[0][:], in_=u3b[0:P, :])
            for k in range(QC):
                nc.sync.dma_start(out=adj2_sb[0][:, k * N:(k + 1) * N],
                                  in_=adjT[k * P:(k + 1) * P, :])
            nc.sync.dma_start(out=wx_sb[:, 0:H * E + N // 2],
                              in_=wx[:, 0:H * E + N // 2])
            nc.sync.dma_start(out=wx_sb[:, H * E + N // 2:WXW],
                              in_=wx[:, H * E + N // 2:WXW])
            for g in range(1, NG):
                for k in range(QC):
                    c = g * QC + k
                    nc.sync.dma_start(out=adj2_sb[g][:, k * N:(k + 1) * N],
                                      in_=adjT[c * P:(c + 1) * P, :])
                if g < H:
                    nc.sync.dma_start(out=u3b_sb[g][:],
                                      in_=u3b[g * P:(g + 1) * P, :])
            for h in range(max(1, NG), H):
                nc.sync.dma_start(out=u3b_sb[h][:],
                                  in_=u3b[h * P:(h + 1) * P, :])

            # ---- PSUM: proj 1 bank + 7 accumulator banks = 8 (bufs is
            # per-tag, so acc_ps holds NCH-1 single-buffered slots).
            with (
                tc.tile_pool(name="proj_ps", bufs=1, space="PSUM") as proj_ps,
                tc.tile_pool(name="acc_ps", bufs=1, space="PSUM") as acc_ps,
            ):
                # projection: all H heads in one rhs (H*E <= 512 cols)
                for c in range(NCH):
                    ps = proj_ps.tile([P, H * E], F32, tag="proj")
                    nc.tensor.matmul(
                        ps[:],
                        xt_sb[:, c * P:(c + 1) * P],
                        kw_sb,
                        start=True, stop=True,
                    )
                    nc.scalar.copy(
                        fs[c][:].rearrange("p (h f) -> p h f", h=H)[:, :, 0:E],
                        ps[:].rearrange("p (h f) -> p h f", h=H))

                # ---- per-head attention ----
                with (
                    tc.tile_pool(name="q", bufs=2) as q_pool,
                    tc.tile_pool(name="p", bufs=3) as p_pool,
                    tc.tile_pool(name="ep", bufs=4) as ep_pool,
                ):
                    HB = NCH // 2

                    def emit_scores(h, g):
                        # fused score + mask ops for group g of head h
                        q2 = q_pool.tile([P, QC * N], F16, tag="q",
                                         name=f"q{h}_{g}")
                        for k in range(QC):
                            c = g * QC + k
                            sc = (c * H + h) * 2
                            nc.vector.tensor_scalar(
                                out=q2[:, k * N:(k + 1) * N],
                                in0=u3b_sb[h][:],
                                scalar1=svm_sb[:, sc:sc + 1],
                                scalar2=svm_sb[:, sc + 1:sc + 2],
                                op0=mybir.AluOpType.mult,
                                op1=mybir.AluOpType.max)
                        p2 = p_pool.tile([P, QC * N], F16, tag="p",
                                         name=f"p{h}_{g}")
                        # split the final group's mask op so the stop-
                        # matmuls (and the epilogue) start half a quad
                        # sooner; the whole last head runs fine-grained to
                        # shorten the kernel tail
                        halves = 2 if QC >= 2 and (
                            g == NG - 1 or h == H - 1) else 1
                        W = QC * N // halves
                        for hv in range(halves):
                            nc.vector.tensor_tensor(
                                p2[:, hv * W:(hv + 1) * W],
                                q2[:, hv * W:(hv + 1) * W],
                                adj2_sb[g][:, hv * W:(hv + 1) * W],
                                mybir.AluOpType.mult)
                        return p2

                    def emit_epilogue(h, accs):
                        # out = relu(acc / rowsum), ones-col rowsum
                        obh = [ep_pool.tile([P, HB * E], F16,
                                            tag=f"obh{half}", bufs=2,
                                            name=f"obh{h}_{half}")
                               for half in range(2)]
                        for ib in range(NCH):
                            acc = accs[ib]
                            rec = ep_pool.tile([P, 1], F32, tag="rec",
                                               name=f"rec{h}_{ib}")
                            nc.vector.reciprocal(rec[:], acc[:, E:E + 1])
                            nc.scalar.activation(
                                obh[ib // HB][:, (ib % HB) * E:
                                              (ib % HB + 1) * E],
                                acc[:, 0:E],
                                mybir.ActivationFunctionType.Relu,
                                bias=0.0, scale=rec[:])
                        # two DMAs per head (i-block halves):
                        # partition r, free (ib, c) -> row ib*P+r, col h*E+c
                        for half in range(2):
                            nc.sync.dma_start(
                                out=out[half * HB * P:(half + 1) * HB * P,
                                        h * E:(h + 1) * E].rearrange(
                                    "(ib r) c -> r ib c", r=P),
                                in_=obh[half][:].rearrange(
                                    "p (ib c) -> p ib c", c=E))

                    pending = None
                    for h in range(H):
                        # Emit the next head's first score group before the
                        # previous head's epilogue: the DVE queue is strict
                        # FIFO, so this keeps score production streaming
                        # across the head boundary instead of stalling
                        # behind reciprocals that wait on stop-matmuls.
                        p_first = emit_scores(h, 0)
                        if pending is not None:
                            emit_epilogue(*pending)
                        # NCH concurrent accumulators: i-blocks 0..NCH-1
                        accs = []
                        for ib in range(NCH):
                            if ib < NCH - 1:
                                accs.append(acc_ps.tile(
                                    [P, EA], F32, tag=f"acc{ib}",
                                    name=f"acc{h}_{ib}"))
                            else:
                                accs.append(proj_ps.tile(
                                    [P, EA], F32, tag="proj",
                                    name=f"acc{h}_{ib}"))
                        for g in range(NG):
                            p2 = p_first if g == 0 else emit_scores(h, g)
                            for k in range(QC):
                                c = g * QC + k
                                for ib in range(NCH):
                                    nc.tensor.matmul(
                                        accs[ib][:],
                                        p2[:, k * N + ib * P:
                                           k * N + (ib + 1) * P],
                                        fs[c][:, h * EA:(h + 1) * EA],
                                        start=(c == 0), stop=(c == NCH - 1),
                                    )
                        pending = (h, accs)
                    emit_epilogue(*pending)
    nc.compile()
    return nc


_PROGRAM_CACHE = {}


def _get_program(N, H):
    key = (N, H)
    if key not in _PROGRAM_CACHE:
        _PROGRAM_CACHE[key] = build_core_program(N, H)
    return _PROGRAM_CACHE[key]


def host_prep(x, adj, kernel, attn_self, attn_neigh):
    """Build per-core input maps (layout transforms + small vector math)."""
    B, N, D = x.shape
    H, _, E = kernel.shape
    NCH = N // P
    kas = np.stack([kernel[h] @ attn_self[h] for h in range(H)], 1)  # [D,H]
    kan = np.stack([kernel[h] @ attn_neigh[h] for h in range(H)], 1)
    kw = np.concatenate([kernel[h] for h in range(H)], axis=1)  # [D, H*E]
    in_maps = []
    for b in range(B):
        a_s = x[b] @ kas   # [N, H]
        a_n = x[b] @ kan
        u3 = np.exp(0.8 * a_s.T)                  # [H, N]
        s23 = np.exp(a_n.T - 2.0)
        v2 = np.exp(0.2 * a_n.T - 2.0)
        u3b = np.repeat(u3.astype(NP16), P, axis=0)   # [H*P, N] broadcast
        # svm[r, (c*H+h)*2 + {0,1}] = {s23, v2}[h, c*128+r]
        sv = np.stack([s23, v2], -1)                  # [H, N, 2]
        svm = sv.transpose(1, 0, 2).reshape(NCH, P, H * 2)
        svm = svm.transpose(1, 0, 2).reshape(P, NCH * H * 2)
        wx = np.concatenate([kw, x[b].T], axis=1)
        in_maps.append({
            "wx": np.ascontiguousarray(wx).astype(NP16),
            "adjT": np.ascontiguousarray(adj[b].T).astype(NP16),
            "u3b": np.ascontiguousarray(u3b),
            "svm": np.ascontiguousarray(svm).astype(np.float32),
        })
    return in_maps


def kernel(x, adj, kernel, attn_self, attn_neigh, bias, _profile=None):
    x = np.asarray(x, np.float32)
    adj = np.asarray(adj, np.float32)
    kernel = np.asarray(kernel, np.float32)
    attn_self = np.asarray(attn_self, np.float32)
    attn_neigh = np.asarray(attn_neigh, np.float32)
    bias = np.asarray(bias, np.float32)

    B, N, D = x.shape
    H, _, E = kernel.shape
    nc = _get_program(N, H)
    in_maps = host_prep(x, adj, kernel, attn_self, attn_neigh)
    kwargs = dict(_profile) if _profile else {}
    last_err = None
    for _attempt in range(3):
        try:
            res = run_bass_kernel_spmd(nc, in_maps, list(range(B)), **kwargs)
            outs = np.stack(
                [np.asarray(res.results[b]["out"]).astype(np.float32)
                 for b in range(B)])
            if np.isnan(outs).any() or np.isinf(outs).any():
                last_err = RuntimeError("non-finite kernel output")
                continue
            break
        except Exception as exc:  # transient PJRT/axon fetch errors
            last_err = exc
    else:
        raise last_err
    assert not np.any(bias != 0.0), "nonzero-bias path not implemented"
    if _profile:
        return outs, res
    return outs


if __name__ == "__main__":
    # Mini smoke test: N=256, H=2, B=2 against a numpy reference.
    np.random.seed(0)
    N, H, D, E, B = 256, 2, 128, 128, 2
    x = np.random.randn(B, N, D).astype(np.float32)
    adj = (np.random.rand(B, N, N) < 0.5).astype(np.float32)
    K = (np.random.randn(H, D, E) / np.sqrt(D)).astype(np.float32)
    a_s = (np.random.randn(H, E) / np.sqrt(E)).astype(np.float32)
    a_n = (np.random.randn(H, E) / np.sqrt(E)).astype(np.float32)
    bias = np.zeros((H, E), np.float32)

    def ref(x, adj, K, a_s, a_n, bias):
        feat = np.einsum('bnd,hde->bhne', x, K)
        s1 = np.einsum('bhne,he->bhn', feat, a_s)
        s2 = np.einsum('bhne,he->bhn', feat, a_n)
        sc = s1[..., :, None] + s2[..., None, :]
        sc = np.where(sc > 0, sc, 0.2 * sc)
        sc = sc + (-1e10) * (1.0 - adj[:, None])
        sc = sc - sc.max(axis=-1, keepdims=True)
        att = np.exp(sc)
        att = att / att.sum(axis=-1, keepdims=True)
        o = np.einsum('bhnm,bhme->bhne', att, feat) + bias[None, :, None, :]
        o = o.transpose(0, 2, 1, 3).reshape(B, N, H * E)
        return np.maximum(o, 0.0)

    expected = ref(x, adj, K, a_s, a_n, bias)
    nc = _get_program(N, H)
    in_maps = host_prep(x, adj, K, a_s, a_n)
    res = run_bass_kernel_spmd(nc, in_maps, list(range(B)))
    actual = np.stack([np.asarray(res.results[b]["out"]).astype(np.float32)
                       for b in range(B)])
    err = np.abs(actual - expected).max() / np.abs(expected).max()
    rel = np.linalg.norm(actual - expected) / np.linalg.norm(expected)
    print(f"SMOKE absmax-rel: {err:.3e}  l2-rel: {rel:.3e}")
